# revision 18
# baseline (speedup 1.0000x reference)
"""Trainium2 Bass kernel for sparse causal attention (nn_CausalAttentionKV).

Reference computation (fp32, single device):
    q_all = x @ Wq + bq ; k_all = x @ Wk + bk ; v_all = x @ Wv + bv
    q = gather(q_all, query_idx)        # (B, M, D) selected query rows
    att = softmax(mask(q k^T / sqrt(hd)))   # per-query causal mask t <= qidx[m]
    y = (att v) @ Wo + bo

Shapes: B=4, T=4096, D=2048, n_head=16, hd=128, M=512.

Sharding (8 cores): core = 2*b + g  handles batch b and head-group g
(8 heads = 1024 feature cols).  Q/K/V projections are column-parallel,
out-proj is row-parallel; the two partial outputs per batch are summed
on the host.  All matmul inputs are bf16 (fp32 PSUM accumulation).

FUSED design (v4): attention is fused into the K/V projection pass,
flash-attention style.  For each 512-key window ts: project V, project
K (8 heads), and as each head's K lands compute its scores, mask, exp
and (two heads later) its P@V.  The ~93us of scalar-engine exp work
(which previously bounded a separate attention phase) hides under the
projection matmul wall, K never round-trips through DRAM, and V is a
small SBUF ring instead of an 8 MB resident tensor.  V-before-K lets
P@V run inside the same window with a 2-head lag, keeping the exp ring
small and the cross-engine dependence acyclic.

Softmax bookkeeping avoids PE work: exp outputs accumulate into a
per-head fp32 e_total on the vector engine; ONE row-sum matmul per
head (vs one per chunk-pair) yields l; 1/l is broadcast across
partitions with a cheap bf16 rank-1 matmul (vs fp32 LOW_HIGH passes),
and those chains ride inside the last window so normalized outputs are
ready the moment the last P@V lands.  The un-normalized P@V partials
accumulate into po_sb on the vector engine (PSUM holds only one
window's partial).

Startup: the Q projection is pipelined by d-chunk (8 PSUM banks
accumulate all 8 heads) while wq/xq/wv/x(window 0) stream just-in-time
in one interleaved DMA stream, so the PE starts ~2us in and window 0
is fed when AQ retires.  Phase C streams wo in 512-col slices from a
small top-level ring whose first two slices load during the pass.
"""

import sys
import types
from contextlib import ExitStack

import numpy as np
import ml_dtypes

import concourse.bass as bass
import concourse.tile as tile
import concourse.mybir as mybir
from concourse import bacc
from concourse.bass_utils import run_bass_kernel_spmd

BF16 = mybir.dt.bfloat16
F32 = mybir.dt.float32
NPBF = ml_dtypes.bfloat16

B, T, D = 4, 4096, 2048
NH, HD, M = 16, 128, 512
NHG = 8            # heads per core (group)
DG = NHG * HD      # 1024 feature cols per core
NT = T // 128      # 32 t-chunks
ND = D // 128      # 16 d-chunks
KTS = 512          # keys per fused iteration
NTS = T // KTS     # 8 fused iterations
MASK_VAL = np.float32(-30000.0)


def _install_ntff_hook():
    """Register the axon NTFF profiling hook if the image's antenv lacks it."""
    try:
        from antenv.axon_hooks import get_axon_ntff_profile_hook  # noqa: F401
        return
    except ImportError:
        pass
    try:
        import antenv
        from trn_agent_boot.trn_boot import _ntff_profile_via_ctypes

        mod = types.ModuleType("antenv.axon_hooks")
        hook = [None]
        mod.set_axon_ntff_profile_hook = lambda h: hook.__setitem__(0, h)
        mod.get_axon_ntff_profile_hook = lambda: hook[0]
        sys.modules["antenv.axon_hooks"] = mod
        antenv.axon_hooks = mod
        mod.set_axon_ntff_profile_hook(
            _ntff_profile_via_ctypes("/opt/axon/libaxon_pjrt.so")
        )
    except Exception:
        pass


def build_program(flo, fhi):
    """Build the per-core Bass program.

    flo[i]: first m column with any allowed key in t-chunk i (cols below
    are fully masked there -> never computed).
    fhi[i]: first m column fully allowed in t-chunk i (cols beyond need
    no mask add).  Both are unions over the 4 batches so one program
    serves all cores.  flo is nondecreasing (qidx sorted per batch).
    """
    nc = bacc.Bacc("TRN2", target_bir_lowering=False, debug=False)

    xT = nc.dram_tensor("xT", [D, T], BF16, kind="ExternalInput")
    xqT = nc.dram_tensor("xqT", [D, M], BF16, kind="ExternalInput")
    wk = nc.dram_tensor("wk", [D, DG], BF16, kind="ExternalInput")
    wv = nc.dram_tensor("wv", [D, DG], BF16, kind="ExternalInput")
    wq = nc.dram_tensor("wq", [D, DG], BF16, kind="ExternalInput")
    wo = nc.dram_tensor("wo", [DG, D], BF16, kind="ExternalInput")
    maskd = nc.dram_tensor("mask", [T, M], BF16, kind="ExternalInput")
    bks = nc.dram_tensor("bks", [128, NHG], F32, kind="ExternalInput")
    bqs = nc.dram_tensor("bqs", [128, NHG], F32, kind="ExternalInput")
    y = nc.dram_tensor("y", [M, D], F32, kind="ExternalOutput")

    # (c*128+p, t) views for chunked DMA
    xTr = xT.rearrange("(c p) t -> p c t", p=128)
    xqTr = xqT.rearrange("(c p) t -> p c t", p=128)
    wkr = wk.rearrange("(c p) t -> p c t", p=128)
    wvr = wv.rearrange("(c p) t -> p c t", p=128)
    wqr = wq.rearrange("(c p) t -> p c t", p=128)
    wor = wo.rearrange("(c p) t -> p c t", p=128)
    maskr = maskd.rearrange("(c p) t -> p c t", p=128)

    # active chunks per ts window (flo nondecreasing -> consecutive prefix)
    def win_chunks(ts):
        return [i for i in range(4 * ts, 4 * ts + 4) if flo[i] < M]

    active_ts = [ts for ts in range(NTS) if win_chunks(ts)]
    last_ts = max(active_ts)
    mlo = [min(flo[4 * g : 4 * g + 4]) for g in range(NTS)]
    mhi = [max(fhi[4 * g : 4 * g + 4]) for g in range(NTS)]
    wmax = max(max(mhi[g] - mlo[g], 1) for g in range(NTS))

    with ExitStack() as ctx:
        tc = ctx.enter_context(tile.TileContext(nc))

        # ---- persistent tiles --------------------------------------
        persist = ctx.enter_context(tc.tile_pool(name="persist", bufs=1))
        # qt[j] holds Q^T until the last scores; the normalized output
        # O^T overwrites it in place at the tail (disjoint lifetimes).
        qt_t = [persist.tile([128, M], BF16, name=f"qt{j}", tag=f"qt{j}") for j in range(NHG)]
        ot_t = qt_t
        etot = [persist.tile([128, M], F32, name=f"et{j}", tag=f"et{j}") for j in range(NHG)]
        po_sb = [persist.tile([128, M], F32, name=f"po{j}", tag=f"po{j}") for j in range(NHG)]
        bias_k = persist.tile([128, NHG], F32, name="bias_k", tag="bias_k")
        bias_q = persist.tile([128, NHG], F32, name="bias_q", tag="bias_q")
        zbias = persist.tile([128, 1], F32, name="zbias", tag="zbias")
        ones_f = persist.tile([128, 128], F32, name="ones_f", tag="ones_f")

        nc.vector.memset(zbias[:], 0.0)
        nc.vector.memset(ones_f[:], 1.0)

        # weight tiles + fused-pass rings (top level: their DMAs ride
        # the AQ stream / early windows)
        wkp = ctx.enter_context(tc.tile_pool(name="wkp", bufs=1))
        wvp = ctx.enter_context(tc.tile_pool(name="wvp", bufs=1))
        wk_t = [wkp.tile([128, 4, DG], BF16, name=f"wk{d}", tag=f"wk{d}") for d in range(4)]
        wv_t = [wvp.tile([128, 4, DG], BF16, name=f"wv{d}", tag=f"wv{d}") for d in range(4)]
        xtp = ctx.enter_context(tc.tile_pool(name="xtp", bufs=2))
        ktp = ctx.enter_context(tc.tile_pool(name="ktp", bufs=2))
        vtp = ctx.enter_context(tc.tile_pool(name="vtp", bufs=4))
        esb = ctx.enter_context(tc.tile_pool(name="esb", bufs=12))
        mkp = ctx.enter_context(tc.tile_pool(name="mkp", bufs=2))
        wop = ctx.enter_context(tc.tile_pool(name="wop", bufs=3))

        # ---- phase AQ: Qt[j] = ((xq @ wq_j + bq_j)/sqrt(hd))^T -----
        # d-chunk pipelined: all 8 heads accumulate in 8 PSUM banks.
        # wq/xq stream just-in-time; wv and window-0 x chunks interleave
        # into the same stream so window 0 is fed when AQ retires.
        xt0 = [xtp.tile([128, 4, KTS], BF16, name=f"xt{d}", tag=f"xt{d}") for d in range(4)]
        with (
            nc.named_scope("phase_AQ"),
            tc.tile_pool(name="wqp", bufs=2) as wqp,
            tc.tile_pool(name="xqp", bufs=2) as xqp,
            tc.tile_pool(name="pq", bufs=1, space="PSUM") as pqp,
        ):
            pq = [pqp.tile([128, M], F32, name=f"pq{j}", tag=f"pq{j}") for j in range(NHG)]
            for d in range(ND):
                wq_d = wqp.tile([128, DG], BF16, name="wqd", tag="wqd")
                nc.sync.dma_start(wq_d[:], wqr[:, d, :])
                xq_d = xqp.tile([128, M], BF16, name="xqd", tag="xqd")
                nc.sync.dma_start(xq_d[:], xqTr[:, d, :])
                nc.sync.dma_start(wv_t[d // 4][:, d % 4, 0:512], wvr[:, d, 0:512])
                nc.sync.dma_start(xt0[d // 4][:, d % 4, :], xTr[:, d, 0:KTS])
                if d == 0:
                    nc.sync.dma_start(bias_q[:], bqs[:])
                    nc.sync.dma_start(bias_k[:], bks[:])
                for j in range(NHG):
                    nc.tensor.matmul(
                        pq[j][:],
                        wq_d[:, j * 128 : (j + 1) * 128],
                        xq_d[:],
                        start=(d == 0),
                        stop=(d == ND - 1),
                        skip_group_check=True,
                    )
            inv_s = 1.0 / float(np.sqrt(HD))
            for j in range(NHG):
                nc.scalar.activation(
                    qt_t[j][:],
                    pq[j][:],
                    mybir.ActivationFunctionType.Identity,
                    scale=inv_s,
                    bias=bias_q[:, j : j + 1],
                )

        # mask(window 0), wv second half, then wk ride during window 0's
        # V section; first wo slices follow
        mk0 = mkp.tile([128, 4, wmax], BF16, name="mk", tag="mk")
        if mlo[0] < M and mhi[0] > mlo[0]:
            nc.sync.dma_start(
                mk0[:, :, : mhi[0] - mlo[0]], maskr[:, 0:4, mlo[0] : mhi[0]]
            )
        for d in range(4):
            nc.sync.dma_start(wv_t[d][:, :, 512:DG], wvr[:, 4 * d : 4 * d + 4, 512:DG])
        for d in range(4):
            nc.sync.dma_start(wk_t[d][:], wkr[:, 4 * d : 4 * d + 4, :])
        wo_s = [wop.tile([128, NHG, 512], BF16, name="wos", tag="wos") for _ in range(3)]
        for fo in range(3):
            nc.sync.dma_start(wo_s[fo][:], wor[:, :, fo * 512 : (fo + 1) * 512])

        # ---- fused pass: V + K projection + attention per window ---
        with (
            nc.named_scope("phase_F"),
            tc.tile_pool(name="kv", bufs=2, space="PSUM") as kvp,
            tc.tile_pool(name="ps", bufs=6, space="PSUM") as psp,
        ):
            et_started = [False] * NHG   # etot[j] initialized?
            po_started = [False] * NHG   # po_sb[j] initialized?

            def emit_scores(j, ts, kt, chunks, mk):
                """Scores+mask+exp for head j on window ts; per-chunk tiles."""
                work = []
                for i in chunks:
                    lo, hi = flo[i], fhi[i]
                    u = i % 4
                    pst = psp.tile([128, M], F32, name="pst", tag="ps")
                    nc.tensor.matmul(
                        pst[:, lo:M],
                        kt[:, u * 128 : (u + 1) * 128],
                        qt_t[j][:, lo:M],
                        start=True,
                        stop=True,
                        skip_group_check=True,
                    )
                    if lo < hi:
                        nc.vector.tensor_add(
                            pst[:, lo:hi],
                            pst[:, lo:hi],
                            mk[:, u, lo - mlo[ts] : hi - mlo[ts]],
                        )
                    e = esb.tile([128, M], BF16, name="e", tag="e")
                    nc.scalar.activation(
                        e[:, lo:M],
                        pst[:, lo:M],
                        mybir.ActivationFunctionType.Exp,
                        bias=zbias[:],
                    )
                    # accumulate softmax denominator on the DVE
                    if not et_started[j]:
                        nc.vector.tensor_copy(etot[j][:, lo:M], e[:, lo:M])
                        if lo > 0:
                            nc.vector.memset(etot[j][:, 0:lo], 0.0)
                        et_started[j] = True
                    else:
                        nc.vector.tensor_add(
                            etot[j][:, lo:M], etot[j][:, lo:M], e[:, lo:M]
                        )
                    work.append((i, e, lo))
                return work

            def emit_pv(j, work, vts, ts):
                """P@V for head j into PSUM, then DVE-accumulate to po_sb.

                PV matmuls run in chunk order (lo nondecreasing), so the
                start=True region [lo0:M] covers every later chunk's
                [lo_i:M] and no PSUM region is read uninitialized.
                """
                work = sorted(work, key=lambda w: w[2])
                lo0 = work[0][2]
                pot = psp.tile([128, M], F32, name="pot", tag="ps")
                for k, (i, e, lo) in enumerate(work):
                    nc.tensor.matmul(
                        pot[:, lo:M],
                        vts[i % 4][:, j * 128 : (j + 1) * 128],
                        e[:, lo:M],
                        start=(k == 0),
                        stop=(k == len(work) - 1),
                        skip_group_check=True,
                    )
                if not po_started[j]:
                    nc.vector.tensor_copy(po_sb[j][:, lo0:M], pot[:, lo0:M])
                    if lo0 > 0:
                        nc.vector.memset(po_sb[j][:, 0:lo0], 0.0)
                    po_started[j] = True
                else:
                    nc.vector.tensor_add(
                        po_sb[j][:, lo0:M], po_sb[j][:, lo0:M], pot[:, lo0:M]
                    )

            # rotated head order: the last-processed heads' l-chains and
            # normalization trail past the final window, so phase C (which
            # accumulates in this same order) reaches them last
            heads = [3, 4, 5, 6, 7, 0, 1, 2]
            first = True
            def emit_norm(j):
                # one fp32 matmul with an all-ones [128,128] stationary
                # computes the key-dim rowsum of etot ALREADY broadcast to
                # all 128 partitions; reciprocal runs full-width in place
                # on PSUM and the normalization multiply reads it directly.
                pbl = psp.tile([128, M], F32, name="pbl", tag="ps")
                nc.tensor.matmul(
                    pbl[:], ones_f[:], etot[j][:],
                    start=True, stop=True, skip_group_check=True,
                )
                nc.vector.reciprocal_approx_fast(pbl[:], pbl[:])
                nc.vector.tensor_mul(ot_t[j][:], po_sb[j][:], pbl[:])

            for ts in range(NTS):
                chunks = win_chunks(ts)
                if not chunks:
                    continue
                if first:
                    xt_t = xt0
                    mk = mk0
                    first = False
                else:
                    xt_t = [xtp.tile([128, 4, KTS], BF16, name=f"xt{d}", tag=f"xt{d}") for d in range(4)]
                    for d in range(4):
                        nc.sync.dma_start(
                            xt_t[d][:], xTr[:, 4 * d : 4 * d + 4, ts * KTS : (ts + 1) * KTS]
                        )
                    mk = mkp.tile([128, 4, wmax], BF16, name="mk", tag="mk")
                    if mlo[ts] < M and mhi[ts] > mlo[ts]:
                        nc.sync.dma_start(
                            mk[:, :, : mhi[ts] - mlo[ts]],
                            maskr[:, 4 * ts : 4 * ts + 4, mlo[ts] : mhi[ts]],
                        )
                # V projection first (f-half outer: half of wv feeds the
                # first 64 matmuls, so window 0 starts at the DMA floor)
                vts = [vtp.tile([128, DG], BF16, name="vt", tag="vt") for _ in range(4)]
                for f in range(2):
                    for u in range(4):
                        pv = kvp.tile([128, 512], F32, name="pv", tag="kv")
                        for d in range(ND):
                            nc.tensor.matmul(
                                pv[:],
                                xt_t[d // 4][:, d % 4, u * 128 : (u + 1) * 128],
                                wv_t[d // 4][:, d % 4, f * 512 : (f + 1) * 512],
                                start=(d == 0),
                                stop=(d == ND - 1),
                            )
                        nc.vector.tensor_copy(vts[u][:, f * 512 : (f + 1) * 512], pv[:])
                # K projection; scores chase the K stream with P@V two
                # heads behind (v tiles already exist)
                kts = {}
                works = {}
                for p in range(NHG):
                    j = heads[p]
                    pk = kvp.tile([128, KTS], F32, name="pk", tag="kv")
                    for d in range(ND):
                        nc.tensor.matmul(
                            pk[:],
                            wk_t[d // 4][:, d % 4, j * 128 : (j + 1) * 128],
                            xt_t[d // 4][:, d % 4, :],
                            start=(d == 0),
                            stop=(d == ND - 1),
                        )
                    kt = ktp.tile([128, KTS], BF16, name="kt", tag="kt")
                    # on the DVE, not scalar: window 0's K section has the
                    # scalar engine saturated by full-width exps
                    nc.vector.tensor_scalar_add(kt[:], pk[:], bias_k[:, j : j + 1])
                    kts[j] = kt
                    if p >= 1:
                        jp = heads[p - 1]
                        works[jp] = emit_scores(jp, ts, kts.pop(jp), chunks, mk)
                        if p >= 3 and heads[p - 3] in works:
                            emit_pv(heads[p - 3], works.pop(heads[p - 3]), vts, ts)
                        if ts == last_ts and p >= 4:
                            emit_norm(heads[p - 4])
                jp = heads[NHG - 1]
                works[jp] = emit_scores(jp, ts, kts.pop(jp), chunks, mk)
                for jp in heads:
                    if jp in works:
                        emit_pv(jp, works.pop(jp), vts, ts)

            # normalization for the heads whose P@V finished at the very
            # end of the last window (the first four rotated heads were
            # normalized inline, mid-window)
            for j in heads[4:]:
                emit_norm(j)

        # ---- phase C: y = O @ wo, wo streamed in 512-col slices ----
        with (
            nc.named_scope("phase_C"),
            tc.tile_pool(name="py", bufs=5, space="PSUM") as pyp,
            tc.tile_pool(name="ysb", bufs=4) as ysb,
        ):
            for fo in range(D // 512):
                for mb in range(M // 128):
                    py = pyp.tile([128, 512], F32, name="py", tag="py")
                    for k, j in enumerate([3, 4, 5, 6, 7, 0, 1, 2]):
                        nc.tensor.matmul(
                            py[:],
                            ot_t[j][:, mb * 128 : (mb + 1) * 128],
                            wo_s[fo % 3][:, j, :],
                            start=(k == 0),
                            stop=(k == NHG - 1),
                        )
                    ys = ysb.tile([128, 512], F32, name="ys", tag="ys")
                    nc.scalar.copy(ys[:], py[:])
                    nc.sync.dma_start(
                        y[
                            mb * 128 : (mb + 1) * 128,
                            fo * 512 : (fo + 1) * 512,
                        ],
                        ys[:],
                    )
                # refill this ring slot with the slice three steps ahead
                if fo + 3 < D // 512:
                    wo_s[fo % 3] = wop.tile([128, NHG, 512], BF16, name="wos", tag="wos")
                    nc.sync.dma_start(
                        wo_s[fo % 3][:],
                        wor[:, :, (fo + 3) * 512 : (fo + 4) * 512],
                    )

    nc.compile()
    return nc


_cache = {}


def _get_program(flo, fhi):
    key = (tuple(flo), tuple(fhi))
    if key not in _cache:
        _cache[key] = build_program(list(flo), list(fhi))
    return _cache[key]


def _prep(inputs):
    x = np.asarray(inputs["x"], dtype=np.float32)
    qidx = np.asarray(inputs["query_idx"]).astype(np.int64)
    Wq = np.asarray(inputs["Wq"], dtype=np.float32)
    Wk = np.asarray(inputs["Wk"], dtype=np.float32)
    Wv = np.asarray(inputs["Wv"], dtype=np.float32)
    Wo = np.asarray(inputs["Wo"], dtype=np.float32)
    bq = np.asarray(inputs["bq"], dtype=np.float32)
    bk = np.asarray(inputs["bk"], dtype=np.float32)
    bv = np.asarray(inputs["bv"], dtype=np.float32)
    bo = np.asarray(inputs["bo"], dtype=np.float32)

    # Per-t-chunk skip bounds, union over batches.  flo[i] = first m that
    # attends into chunk i (everything below is fully masked there);
    # fhi[i] = one past the last m only partially covered by chunk i.
    # Computed positionally so they are correct even for unsorted
    # query_idx (just less effective at skipping).
    flo = [M] * NT
    fhi = [0] * NT
    for b in range(B):
        for i in range(NT):
            allowed = qidx[b] >= 128 * i          # chunk i not fully masked
            partial = qidx[b] < 128 * (i + 1)     # chunk i not fully allowed
            lo_b = int(np.argmax(allowed)) if allowed.any() else M
            hi_b = M - int(np.argmax(partial[::-1])) if partial.any() else 0
            flo[i] = min(flo[i], lo_b)
            fhi[i] = max(fhi[i], hi_b)

    in_maps = []
    tgrid = np.arange(T)[:, None]
    for core in range(8):
        b, g = divmod(core, 2)
        sl = slice(g * DG, (g + 1) * DG)
        xb = x[b]
        mask = np.where(tgrid <= qidx[b][None, :], np.float32(0), MASK_VAL)
        in_maps.append(
            {
                "xT": np.ascontiguousarray(xb.T.astype(NPBF)),
                "xqT": np.ascontiguousarray(xb[qidx[b]].T.astype(NPBF)),
                "wk": np.ascontiguousarray(Wk[:, sl].astype(NPBF)),
                "wv": np.ascontiguousarray(Wv[:, sl].astype(NPBF)),
                "wq": np.ascontiguousarray(Wq[:, sl].astype(NPBF)),
                "wo": np.ascontiguousarray(Wo[sl, :].astype(NPBF)),
                "mask": np.ascontiguousarray(mask.astype(NPBF)),
                "bks": np.ascontiguousarray(bk[sl].reshape(NHG, 128).T),
                "bqs": np.ascontiguousarray(
                    (bq[sl] / np.sqrt(HD)).reshape(NHG, 128).T.astype(np.float32)
                ),
            }
        )

    const = (bv.astype(np.float64) @ Wo.astype(np.float64) + bo).astype(np.float32)
    return flo, fhi, in_maps, const


def run(inputs, trace=False, trace_kwargs=None):
    _install_ntff_hook()
    flo, fhi, in_maps, const = _prep(inputs)
    nc = _get_program(flo, fhi)
    res = run_bass_kernel_spmd(
        nc, in_maps, list(range(8)), trace=trace, **(trace_kwargs or {})
    )
    out = np.zeros((B, M, D), dtype=np.float32)
    for b in range(B):
        out[b] = res.results[2 * b]["y"] + res.results[2 * b + 1]["y"] + const
    return out, res


def kernel(**inputs) -> np.ndarray:
    out, _ = run(inputs, trace=False)
    return out


# revision 19
# speedup vs baseline: 1.1833x; 1.1833x over previous
"""Trainium2 Bass kernel for sparse causal attention (nn_CausalAttentionKV).

Reference computation (fp32, single device):
    q_all = x @ Wq + bq ; k_all = x @ Wk + bk ; v_all = x @ Wv + bv
    q = gather(q_all, query_idx)        # (B, M, D) selected query rows
    att = softmax(mask(q k^T / sqrt(hd)))   # per-query causal mask t <= qidx[m]
    y = (att v) @ Wo + bo

Shapes: B=4, T=4096, D=2048, n_head=16, hd=128, M=512.

Sharding (8 cores): core = 2*b + g  handles batch b and head-group g
(8 heads = 1024 feature cols).  Q/K/V projections are column-parallel,
out-proj is row-parallel; the two partial outputs per batch are summed
on the host.  All matmul inputs are bf16 (fp32 PSUM accumulation).

FUSED design (v4): attention is fused into the K/V projection pass,
flash-attention style.  For each 512-key window ts: project V, project
K (8 heads), and as each head's K lands compute its scores, mask, exp
and (two heads later) its P@V.  The ~93us of scalar-engine exp work
(which previously bounded a separate attention phase) hides under the
projection matmul wall, K never round-trips through DRAM, and V is a
small SBUF ring instead of an 8 MB resident tensor.  V-before-K lets
P@V run inside the same window with a 2-head lag, keeping the exp ring
small and the cross-engine dependence acyclic.

Softmax bookkeeping avoids PE work: exp outputs accumulate into a
per-head fp32 e_total on the vector engine; ONE row-sum matmul per
head (vs one per chunk-pair) yields l; 1/l is broadcast across
partitions with a cheap bf16 rank-1 matmul (vs fp32 LOW_HIGH passes),
and those chains ride inside the last window so normalized outputs are
ready the moment the last P@V lands.  The un-normalized P@V partials
accumulate into po_sb on the vector engine (PSUM holds only one
window's partial).

Startup: the Q projection is pipelined by d-chunk (8 PSUM banks
accumulate all 8 heads) while wq/xq/wv/x(window 0) stream just-in-time
in one interleaved DMA stream, so the PE starts ~2us in and window 0
is fed when AQ retires.  Phase C streams wo in 512-col slices from a
small top-level ring whose first two slices load during the pass.
"""

import sys
import types
from contextlib import ExitStack

import numpy as np
import ml_dtypes

import concourse.bass as bass
import concourse.tile as tile
import concourse.mybir as mybir
from concourse import bacc
from concourse.bass_utils import run_bass_kernel_spmd

BF16 = mybir.dt.bfloat16
F32 = mybir.dt.float32
NPBF = ml_dtypes.bfloat16

B, T, D = 4, 4096, 2048
NH, HD, M = 16, 128, 512
NHG = 8            # heads per core (group)
DG = NHG * HD      # 1024 feature cols per core
NT = T // 128      # 32 t-chunks
ND = D // 128      # 16 d-chunks
KTS = 512          # keys per fused iteration
NTS = T // KTS     # 8 fused iterations
MASK_VAL = np.float32(-30000.0)


def _install_ntff_hook():
    """Register the axon NTFF profiling hook if the image's antenv lacks it."""
    try:
        from antenv.axon_hooks import get_axon_ntff_profile_hook  # noqa: F401
        return
    except ImportError:
        pass
    try:
        import antenv
        from trn_agent_boot.trn_boot import _ntff_profile_via_ctypes

        mod = types.ModuleType("antenv.axon_hooks")
        hook = [None]
        mod.set_axon_ntff_profile_hook = lambda h: hook.__setitem__(0, h)
        mod.get_axon_ntff_profile_hook = lambda: hook[0]
        sys.modules["antenv.axon_hooks"] = mod
        antenv.axon_hooks = mod
        mod.set_axon_ntff_profile_hook(
            _ntff_profile_via_ctypes("/opt/axon/libaxon_pjrt.so")
        )
    except Exception:
        pass


def build_program(flo, fhi):
    """Build the per-core Bass program.

    flo[i]: first m column with any allowed key in t-chunk i (cols below
    are fully masked there -> never computed).
    fhi[i]: first m column fully allowed in t-chunk i (cols beyond need
    no mask add).  Both are unions over the 4 batches so one program
    serves all cores.  flo is nondecreasing (qidx sorted per batch).
    """
    nc = bacc.Bacc("TRN2", target_bir_lowering=False, debug=False)

    xT = nc.dram_tensor("xT", [D, T], BF16, kind="ExternalInput")
    xqT = nc.dram_tensor("xqT", [D, M], BF16, kind="ExternalInput")
    wk = nc.dram_tensor("wk", [D, DG], BF16, kind="ExternalInput")
    wv = nc.dram_tensor("wv", [D, DG], BF16, kind="ExternalInput")
    wq = nc.dram_tensor("wq", [D, DG], BF16, kind="ExternalInput")
    wo = nc.dram_tensor("wo", [DG, D], BF16, kind="ExternalInput")
    maskd = nc.dram_tensor("mask", [T, M], BF16, kind="ExternalInput")
    bks = nc.dram_tensor("bks", [128, NHG], F32, kind="ExternalInput")
    bqs = nc.dram_tensor("bqs", [128, NHG], F32, kind="ExternalInput")
    y = nc.dram_tensor("y", [M, D], F32, kind="ExternalOutput")

    # (c*128+p, t) views for chunked DMA
    xTr = xT.rearrange("(c p) t -> p c t", p=128)
    xqTr = xqT.rearrange("(c p) t -> p c t", p=128)
    wkr = wk.rearrange("(c p) t -> p c t", p=128)
    wvr = wv.rearrange("(c p) t -> p c t", p=128)
    wqr = wq.rearrange("(c p) t -> p c t", p=128)
    wor = wo.rearrange("(c p) t -> p c t", p=128)
    maskr = maskd.rearrange("(c p) t -> p c t", p=128)

    # active chunks per ts window (flo nondecreasing -> consecutive prefix)
    def win_chunks(ts):
        return [i for i in range(4 * ts, 4 * ts + 4) if flo[i] < M]

    active_ts = [ts for ts in range(NTS) if win_chunks(ts)]
    last_ts = max(active_ts)
    mlo = [min(flo[4 * g : 4 * g + 4]) for g in range(NTS)]
    mhi = [max(fhi[4 * g : 4 * g + 4]) for g in range(NTS)]
    wmax = max(max(mhi[g] - mlo[g], 1) for g in range(NTS))

    with ExitStack() as ctx:
        tc = ctx.enter_context(tile.TileContext(nc))

        # ---- persistent tiles --------------------------------------
        persist = ctx.enter_context(tc.tile_pool(name="persist", bufs=1))
        # qt[j] holds Q^T until the last scores; the normalized output
        # O^T overwrites it in place at the tail (disjoint lifetimes).
        qt_t = [persist.tile([128, M], BF16, name=f"qt{j}", tag=f"qt{j}") for j in range(NHG)]
        ot_t = qt_t
        etot = [persist.tile([128, M], F32, name=f"et{j}", tag=f"et{j}") for j in range(NHG)]
        po_sb = [persist.tile([128, M], F32, name=f"po{j}", tag=f"po{j}") for j in range(NHG)]
        bias_k = persist.tile([128, NHG], F32, name="bias_k", tag="bias_k")
        bias_q = persist.tile([128, NHG], F32, name="bias_q", tag="bias_q")
        zbias = persist.tile([128, 1], F32, name="zbias", tag="zbias")
        ones_f = persist.tile([128, 128], F32, name="ones_f", tag="ones_f")

        nc.vector.memset(zbias[:], 0.0)
        nc.vector.memset(ones_f[:], 1.0)

        # weight tiles + fused-pass rings (top level: their DMAs ride
        # the AQ stream / early windows)
        wkp = ctx.enter_context(tc.tile_pool(name="wkp", bufs=1))
        wvp = ctx.enter_context(tc.tile_pool(name="wvp", bufs=1))
        wk_t = [wkp.tile([128, 4, DG], BF16, name=f"wk{d}", tag=f"wk{d}") for d in range(4)]
        wv_t = [wvp.tile([128, 4, DG], BF16, name=f"wv{d}", tag=f"wv{d}") for d in range(4)]
        xtp = ctx.enter_context(tc.tile_pool(name="xtp", bufs=2))
        ktp = ctx.enter_context(tc.tile_pool(name="ktp", bufs=3))
        vtp = ctx.enter_context(tc.tile_pool(name="vtp", bufs=5))
        esb = ctx.enter_context(tc.tile_pool(name="esb", bufs=12))
        mkp = ctx.enter_context(tc.tile_pool(name="mkp", bufs=2))
        wop = ctx.enter_context(tc.tile_pool(name="wop", bufs=3))

        # ---- phase AQ: Qt[j] = ((xq @ wq_j + bq_j)/sqrt(hd))^T -----
        # d-chunk pipelined: all 8 heads accumulate in 8 PSUM banks.
        # wq/xq stream just-in-time; wv and window-0 x chunks interleave
        # into the same stream so window 0 is fed when AQ retires.
        xt0 = [xtp.tile([128, 4, KTS], BF16, name=f"xt{d}", tag=f"xt{d}") for d in range(4)]
        with (
            nc.named_scope("phase_AQ"),
            tc.tile_pool(name="wqp", bufs=2) as wqp,
            tc.tile_pool(name="xqp", bufs=2) as xqp,
            tc.tile_pool(name="pq", bufs=1, space="PSUM") as pqp,
        ):
            pq = [pqp.tile([128, M], F32, name=f"pq{j}", tag=f"pq{j}") for j in range(NHG)]
            for d in range(ND):
                wq_d = wqp.tile([128, DG], BF16, name="wqd", tag="wqd")
                nc.sync.dma_start(wq_d[:], wqr[:, d, :])
                xq_d = xqp.tile([128, M], BF16, name="xqd", tag="xqd")
                nc.sync.dma_start(xq_d[:], xqTr[:, d, :])
                nc.sync.dma_start(wv_t[d // 4][:, d % 4, 0:512], wvr[:, d, 0:512])
                nc.sync.dma_start(xt0[d // 4][:, d % 4, :], xTr[:, d, 0:KTS])
                if d == 0:
                    nc.sync.dma_start(bias_q[:], bqs[:])
                    nc.sync.dma_start(bias_k[:], bks[:])
                for j in range(NHG):
                    nc.tensor.matmul(
                        pq[j][:],
                        wq_d[:, j * 128 : (j + 1) * 128],
                        xq_d[:],
                        start=(d == 0),
                        stop=(d == ND - 1),
                        skip_group_check=True,
                    )
            inv_s = 1.0 / float(np.sqrt(HD))
            for j in range(NHG):
                nc.scalar.activation(
                    qt_t[j][:],
                    pq[j][:],
                    mybir.ActivationFunctionType.Identity,
                    scale=inv_s,
                    bias=bias_q[:, j : j + 1],
                )

        # mask(window 0), wv second half, then wk ride during window 0's
        # V section; first wo slices follow
        mk0 = mkp.tile([128, 4, wmax], BF16, name="mk", tag="mk")
        if mlo[0] < M and mhi[0] > mlo[0]:
            nc.sync.dma_start(
                mk0[:, :, : mhi[0] - mlo[0]], maskr[:, 0:4, mlo[0] : mhi[0]]
            )
        for d in range(4):
            nc.sync.dma_start(wv_t[d][:, :, 512:DG], wvr[:, 4 * d : 4 * d + 4, 512:DG])
        for d in range(4):
            nc.sync.dma_start(wk_t[d][:], wkr[:, 4 * d : 4 * d + 4, :])
        wo_s = [wop.tile([128, NHG, 512], BF16, name="wos", tag="wos") for _ in range(3)]
        for fo in range(3):
            nc.sync.dma_start(wo_s[fo][:], wor[:, :, fo * 512 : (fo + 1) * 512])

        # ---- fused pass: V + K projection + attention per window ---
        with (
            nc.named_scope("phase_F"),
            tc.tile_pool(name="kv", bufs=2, space="PSUM") as kvp,
            tc.tile_pool(name="ps", bufs=6, space="PSUM") as psp,
        ):
            et_started = [False] * NHG   # etot[j] initialized?
            po_started = [False] * NHG   # po_sb[j] initialized?

            def emit_scores(j, ts, kt, chunks, mk):
                """Scores+mask+exp for head j on window ts; per-chunk tiles."""
                work = []
                for i in chunks:
                    lo, hi = flo[i], fhi[i]
                    u = i % 4
                    pst = psp.tile([128, M], F32, name="pst", tag="ps")
                    nc.tensor.matmul(
                        pst[:, lo:M],
                        kt[:, u * 128 : (u + 1) * 128],
                        qt_t[j][:, lo:M],
                        start=True,
                        stop=True,
                        skip_group_check=True,
                    )
                    if lo < hi:
                        nc.vector.tensor_add(
                            pst[:, lo:hi],
                            pst[:, lo:hi],
                            mk[:, u, lo - mlo[ts] : hi - mlo[ts]],
                        )
                    e = esb.tile([128, M], BF16, name="e", tag="e")
                    nc.scalar.activation(
                        e[:, lo:M],
                        pst[:, lo:M],
                        mybir.ActivationFunctionType.Exp,
                        bias=zbias[:],
                    )
                    # accumulate softmax denominator on the DVE
                    if not et_started[j]:
                        nc.vector.tensor_copy(etot[j][:, lo:M], e[:, lo:M])
                        if lo > 0:
                            nc.vector.memset(etot[j][:, 0:lo], 0.0)
                        et_started[j] = True
                    else:
                        nc.vector.tensor_add(
                            etot[j][:, lo:M], etot[j][:, lo:M], e[:, lo:M]
                        )
                    work.append((i, e, lo))
                return work

            def emit_pv(j, work, vts, ts):
                """P@V for head j into PSUM, then DVE-accumulate to po_sb.

                PV matmuls run in chunk order (lo nondecreasing), so the
                start=True region [lo0:M] covers every later chunk's
                [lo_i:M] and no PSUM region is read uninitialized.
                """
                work = sorted(work, key=lambda w: w[2])
                lo0 = work[0][2]
                pot = psp.tile([128, M], F32, name="pot", tag="ps")
                for k, (i, e, lo) in enumerate(work):
                    nc.tensor.matmul(
                        pot[:, lo:M],
                        vts[i % 4][:, j * 128 : (j + 1) * 128],
                        e[:, lo:M],
                        start=(k == 0),
                        stop=(k == len(work) - 1),
                        skip_group_check=True,
                    )
                if not po_started[j]:
                    nc.vector.tensor_copy(po_sb[j][:, lo0:M], pot[:, lo0:M])
                    if lo0 > 0:
                        nc.vector.memset(po_sb[j][:, 0:lo0], 0.0)
                    po_started[j] = True
                else:
                    nc.vector.tensor_add(
                        po_sb[j][:, lo0:M], po_sb[j][:, lo0:M], pot[:, lo0:M]
                    )

            # rotated head order: the last-processed heads' l-chains and
            # normalization trail past the final window, so phase C (which
            # accumulates in this same order) reaches them last
            heads = [3, 4, 5, 6, 7, 0, 1, 2]
            first = True
            def emit_norm(j):
                # one fp32 matmul with an all-ones [128,128] stationary
                # computes the key-dim rowsum of etot ALREADY broadcast to
                # all 128 partitions; reciprocal runs full-width in place
                # on PSUM and the normalization multiply reads it directly.
                pbl = psp.tile([128, M], F32, name="pbl", tag="ps")
                nc.tensor.matmul(
                    pbl[:], ones_f[:], etot[j][:],
                    start=True, stop=True, skip_group_check=True,
                )
                nc.vector.reciprocal_approx_fast(pbl[:], pbl[:])
                nc.vector.tensor_mul(ot_t[j][:], po_sb[j][:], pbl[:])

            for ts in range(NTS):
                chunks = win_chunks(ts)
                if not chunks:
                    continue
                if first:
                    xt_t = xt0
                    mk = mk0
                    first = False
                else:
                    xt_t = [xtp.tile([128, 4, KTS], BF16, name=f"xt{d}", tag=f"xt{d}") for d in range(4)]
                    for d in range(4):
                        nc.sync.dma_start(
                            xt_t[d][:], xTr[:, 4 * d : 4 * d + 4, ts * KTS : (ts + 1) * KTS]
                        )
                    mk = mkp.tile([128, 4, wmax], BF16, name="mk", tag="mk")
                    if mlo[ts] < M and mhi[ts] > mlo[ts]:
                        nc.sync.dma_start(
                            mk[:, :, : mhi[ts] - mlo[ts]],
                            maskr[:, 4 * ts : 4 * ts + 4, mlo[ts] : mhi[ts]],
                        )
                # V projection first (f-half outer: half of wv feeds the
                # first 64 matmuls, so window 0 starts at the DMA floor)
                vts = [vtp.tile([128, DG], BF16, name="vt", tag="vt") for _ in range(4)]
                for f in range(2):
                    for u in range(4):
                        pv = kvp.tile([128, 512], F32, name="pv", tag="kv")
                        for d in range(ND):
                            nc.tensor.matmul(
                                pv[:],
                                xt_t[d // 4][:, d % 4, u * 128 : (u + 1) * 128],
                                wv_t[d // 4][:, d % 4, f * 512 : (f + 1) * 512],
                                start=(d == 0),
                                stop=(d == ND - 1),
                            )
                        nc.vector.tensor_copy(vts[u][:, f * 512 : (f + 1) * 512], pv[:])
                # K projection; scores chase the K stream with P@V two
                # heads behind (v tiles already exist)
                kts = {}
                works = {}
                for p in range(NHG):
                    j = heads[p]
                    pk = kvp.tile([128, KTS], F32, name="pk", tag="kv")
                    for d in range(ND):
                        nc.tensor.matmul(
                            pk[:],
                            wk_t[d // 4][:, d % 4, j * 128 : (j + 1) * 128],
                            xt_t[d // 4][:, d % 4, :],
                            start=(d == 0),
                            stop=(d == ND - 1),
                        )
                    kt = ktp.tile([128, KTS], BF16, name="kt", tag="kt")
                    # on the DVE, not scalar: window 0's K section has the
                    # scalar engine saturated by full-width exps
                    nc.vector.tensor_scalar_add(kt[:], pk[:], bias_k[:, j : j + 1])
                    kts[j] = kt
                    if p >= 1:
                        jp = heads[p - 1]
                        works[jp] = emit_scores(jp, ts, kts.pop(jp), chunks, mk)
                        if p >= 3 and heads[p - 3] in works:
                            emit_pv(heads[p - 3], works.pop(heads[p - 3]), vts, ts)
                        if ts == last_ts and p >= 4:
                            emit_norm(heads[p - 4])
                jp = heads[NHG - 1]
                works[jp] = emit_scores(jp, ts, kts.pop(jp), chunks, mk)
                for jp in heads:
                    if jp in works:
                        emit_pv(jp, works.pop(jp), vts, ts)

            # normalization for the heads whose P@V finished at the very
            # end of the last window (the first four rotated heads were
            # normalized inline, mid-window)
            for j in heads[4:]:
                emit_norm(j)

        # ---- phase C: y = O @ wo, wo streamed in 512-col slices ----
        with (
            nc.named_scope("phase_C"),
            tc.tile_pool(name="py", bufs=5, space="PSUM") as pyp,
            tc.tile_pool(name="ysb", bufs=4) as ysb,
        ):
            for fo in range(D // 512):
                for mb in range(M // 128):
                    py = pyp.tile([128, 512], F32, name="py", tag="py")
                    for k, j in enumerate([3, 4, 5, 6, 7, 0, 1, 2]):
                        nc.tensor.matmul(
                            py[:],
                            ot_t[j][:, mb * 128 : (mb + 1) * 128],
                            wo_s[fo % 3][:, j, :],
                            start=(k == 0),
                            stop=(k == NHG - 1),
                        )
                    ys = ysb.tile([128, 512], F32, name="ys", tag="ys")
                    nc.scalar.copy(ys[:], py[:])
                    nc.sync.dma_start(
                        y[
                            mb * 128 : (mb + 1) * 128,
                            fo * 512 : (fo + 1) * 512,
                        ],
                        ys[:],
                    )
                # refill this ring slot with the slice three steps ahead
                if fo + 3 < D // 512:
                    wo_s[fo % 3] = wop.tile([128, NHG, 512], BF16, name="wos", tag="wos")
                    nc.sync.dma_start(
                        wo_s[fo % 3][:],
                        wor[:, :, (fo + 3) * 512 : (fo + 4) * 512],
                    )

    nc.compile()
    return nc


_cache = {}


def _get_program(flo, fhi):
    key = (tuple(flo), tuple(fhi))
    if key not in _cache:
        _cache[key] = build_program(list(flo), list(fhi))
    return _cache[key]


def _prep(inputs):
    x = np.asarray(inputs["x"], dtype=np.float32)
    qidx = np.asarray(inputs["query_idx"]).astype(np.int64)
    Wq = np.asarray(inputs["Wq"], dtype=np.float32)
    Wk = np.asarray(inputs["Wk"], dtype=np.float32)
    Wv = np.asarray(inputs["Wv"], dtype=np.float32)
    Wo = np.asarray(inputs["Wo"], dtype=np.float32)
    bq = np.asarray(inputs["bq"], dtype=np.float32)
    bk = np.asarray(inputs["bk"], dtype=np.float32)
    bv = np.asarray(inputs["bv"], dtype=np.float32)
    bo = np.asarray(inputs["bo"], dtype=np.float32)

    # Per-t-chunk skip bounds, union over batches.  flo[i] = first m that
    # attends into chunk i (everything below is fully masked there);
    # fhi[i] = one past the last m only partially covered by chunk i.
    # Computed positionally so they are correct even for unsorted
    # query_idx (just less effective at skipping).
    flo = [M] * NT
    fhi = [0] * NT
    for b in range(B):
        for i in range(NT):
            allowed = qidx[b] >= 128 * i          # chunk i not fully masked
            partial = qidx[b] < 128 * (i + 1)     # chunk i not fully allowed
            lo_b = int(np.argmax(allowed)) if allowed.any() else M
            hi_b = M - int(np.argmax(partial[::-1])) if partial.any() else 0
            flo[i] = min(flo[i], lo_b)
            fhi[i] = max(fhi[i], hi_b)

    in_maps = []
    tgrid = np.arange(T)[:, None]
    for core in range(8):
        b, g = divmod(core, 2)
        sl = slice(g * DG, (g + 1) * DG)
        xb = x[b]
        mask = np.where(tgrid <= qidx[b][None, :], np.float32(0), MASK_VAL)
        in_maps.append(
            {
                "xT": np.ascontiguousarray(xb.T.astype(NPBF)),
                "xqT": np.ascontiguousarray(xb[qidx[b]].T.astype(NPBF)),
                "wk": np.ascontiguousarray(Wk[:, sl].astype(NPBF)),
                "wv": np.ascontiguousarray(Wv[:, sl].astype(NPBF)),
                "wq": np.ascontiguousarray(Wq[:, sl].astype(NPBF)),
                "wo": np.ascontiguousarray(Wo[sl, :].astype(NPBF)),
                "mask": np.ascontiguousarray(mask.astype(NPBF)),
                "bks": np.ascontiguousarray(bk[sl].reshape(NHG, 128).T),
                "bqs": np.ascontiguousarray(
                    (bq[sl] / np.sqrt(HD)).reshape(NHG, 128).T.astype(np.float32)
                ),
            }
        )

    const = (bv.astype(np.float64) @ Wo.astype(np.float64) + bo).astype(np.float32)
    return flo, fhi, in_maps, const


def run(inputs, trace=False, trace_kwargs=None):
    _install_ntff_hook()
    flo, fhi, in_maps, const = _prep(inputs)
    nc = _get_program(flo, fhi)
    res = run_bass_kernel_spmd(
        nc, in_maps, list(range(8)), trace=trace, **(trace_kwargs or {})
    )
    out = np.zeros((B, M, D), dtype=np.float32)
    for b in range(B):
        out[b] = res.results[2 * b]["y"] + res.results[2 * b + 1]["y"] + const
    return out, res


def kernel(**inputs) -> np.ndarray:
    out, _ = run(inputs, trace=False)
    return out
